# revision 1
# baseline (speedup 1.0000x reference)
"""BCSR GraphConv kernel for 8x Trainium2 NeuronCores.

Computes: out = segment_sum((X @ Wn)[edge_col] * edge_vals, edge_row) + X @ Ws

Strategy (destination-stationary; memory roofline = the edge gather):
  - Nodes are sharded 8 ways (12500 rows per core); edges are partitioned by
    destination row (edge_row is sorted, so each core owns a contiguous edge
    range). The 128x128 weights are replicated.
  - Features are uploaded as an fp16 hi/lo pair table [N, 256] (512-byte rows
    gather at full DMA line rate; hi+lo preserves ~fp32 feature precision
    while keeping PE matmuls at fp16 rate).
  - Edge rows are fetched with bulk dma_gather (int16 indices), so nodes are
    split into 4 buckets of <=25000 rows; each destination tile of 128 nodes
    has a uniform per-bucket block budget (SPMD: one program for all cores).
    Padded slots re-gather the previous real row (HBM page-friendly) with
    val=0 so they contribute nothing. Gather calls are capped at 1024
    indices (SWDGE ring) and round-robin over 4 SWDGE queues — the queues
    parallelize SDMA packet processing (measured 4x gather throughput).
  - Per 128-edge block, a scaled one-hot selection matrix
    onehot[e, d] = (row_local[e] == d) * val[e]
    is built with one fused tensor_scalar (is_equal, mult) op, and the
    segment-sum runs on the TensorEngine as PSUM-accumulated matmuls:
    A[d, 0:256] += onehot.T @ G_pair   (hi|lo halves accumulate in one op).
  - Tail per tile: A = A_hi + A_lo (fp32), PE-transpose, then
    out_tile = A @ Wn + X_tile @ Ws in fp32 (X^T is loaded from a
    host-transposed feature table; no on-device transpose of X).
"""

import sys

if "/opt/trn_rl_repo" not in sys.path:
    sys.path.insert(0, "/opt/trn_rl_repo")

import numpy as np

import concourse.bacc as bacc
import concourse.mybir as mybir
import concourse.tile as tile
from concourse.bass_utils import run_bass_kernel_spmd
from concourse.masks import make_identity

D = 128
P = 128
MAX_CALL = 1024  # SWDGE ring capacity per dma_gather

F16 = mybir.dt.float16
F32 = mybir.dt.float32
I16 = mybir.dt.int16
I32 = mybir.dt.int32


def plan_groups(n_tiles, gt):
    groups = []
    t = 0
    while t < n_tiles:
        groups.append((t, min(gt, n_tiles - t)))
        t += min(gt, n_tiles - t)
    return groups


def build_program(n_cores, n_tiles, nbk, rows_last, v_rows, bucket_rows, gt,
                  repeat=1, mode="full", iota32=False):
    """One SPMD program for all cores.

    nbk: block budget (128-edge blocks) per (dest-tile, bucket)
    v_rows: feature-pair table rows;  bucket_rows: rows per int16 bucket
    gt: dest tiles per gather group
    repeat: run the whole body this many times (benchmarking aid)
    """
    n_buckets = (v_rows + bucket_rows - 1) // bucket_rows
    nb = n_buckets * nbk  # blocks per tile
    n_loc = (n_tiles - 1) * P + rows_last
    nblk_total = n_tiles * nb
    slots_total = nblk_total * P
    groups = plan_groups(n_tiles, gt)

    nc = bacc.Bacc(
        "TRN2", target_bir_lowering=False, debug=False, num_devices=n_cores,
        num_swdge_queues=4,
    )

    fpair = nc.dram_tensor("fpair", [v_rows, 2 * D], F16, kind="ExternalInput")
    ftT = nc.dram_tensor("ftT", [D, n_tiles * P], F32, kind="ExternalInput")
    idxm = nc.dram_tensor("idxm", [P, slots_total // 16], I16, kind="ExternalInput")
    rowm = nc.dram_tensor("rowm", [P, nblk_total], F32, kind="ExternalInput")
    valm = nc.dram_tensor("valm", [P, nblk_total], F32, kind="ExternalInput")
    wn = nc.dram_tensor("wn", [D, D], F32, kind="ExternalInput")
    ws = nc.dram_tensor("ws", [D, D], F32, kind="ExternalInput")
    out = nc.dram_tensor("out", [n_loc, D], F32, kind="ExternalOutput")

    with tile.TileContext(nc) as tc:
        with (
            tc.tile_pool(name="const", bufs=1) as cpool,
            tc.tile_pool(name="gather", bufs=2) as gpool,
            tc.tile_pool(name="onehot", bufs=24) as ohpool,
            tc.tile_pool(name="xt", bufs=3) as xtpool,
            tc.tile_pool(name="asb", bufs=3) as apool,
            tc.tile_pool(name="atsb", bufs=3) as atpool,
            tc.tile_pool(name="osb", bufs=3) as opool,
            tc.tile_pool(name="psA", bufs=2, space="PSUM") as psa_pool,
            tc.tile_pool(name="psT", bufs=2, space="PSUM") as pst_pool,
            tc.tile_pool(name="psO", bufs=2, space="PSUM") as pso_pool,
        ):
            idx_sb = cpool.tile([P, slots_total // 16], I16, tag="idx")
            rowm_sb = cpool.tile([P, nblk_total], F32, tag="rowm")
            valm_sb = cpool.tile([P, nblk_total], F32, tag="valm")
            wn_sb = cpool.tile([D, D], F32, tag="wn")
            ws_sb = cpool.tile([D, D], F32, tag="ws")
            ident = cpool.tile([P, P], F32, tag="ident")
            iota_i = cpool.tile([P, P], I32, tag="iota_i")
            iota_h = cpool.tile([P, P], F32 if iota32 else F16, tag="iota_h")

            nc.sync.dma_start(idx_sb[:], idxm[:])
            nc.sync.dma_start(rowm_sb[:], rowm[:])
            nc.sync.dma_start(valm_sb[:], valm[:])
            nc.sync.dma_start(wn_sb[:], wn[:])
            nc.sync.dma_start(ws_sb[:], ws[:])
            make_identity(nc, ident[:])
            nc.gpsimd.iota(iota_i[:], pattern=[[1, P]], base=0, channel_multiplier=0)
            nc.vector.tensor_copy(iota_h[:], iota_i[:])

            def emit_body():
                blk_base = 0  # global block id of current group's first block
                qn = 0
                for t0, gts in groups:
                    g_nblk = gts * nb
                    g = gpool.tile([P, g_nblk, 2 * D], F16, tag="g")
                    for k in range(n_buckets):
                        b_lo = k * bucket_rows
                        b_hi = min(b_lo + bucket_rows, v_rows)
                        run_blk0 = k * gts * nbk
                        run_slots = gts * nbk * P
                        off = 0
                        while off < run_slots:
                            n_i = min(MAX_CALL, run_slots - off)
                            blk0 = run_blk0 + off // P
                            s_abs = (blk_base + run_blk0) * P + off
                            nc.gpsimd.dma_gather(
                                g[:, blk0 : blk0 + n_i // P, :],
                                fpair[b_lo:b_hi, :],
                                idx_sb[:, s_abs // 16 : (s_abs + n_i) // 16],
                                n_i,
                                n_i,
                                2 * D,
                                queue_num=qn % 4,
                            )
                            qn += 1
                            off += n_i

                    for ti in range(gts):
                        if mode == "gather":
                            continue
                        t = t0 + ti
                        psa = psa_pool.tile([P, 2 * D], F32, tag="psa")
                        mm = 0
                        for k in range(n_buckets):
                            for j in range(nbk):
                                rel = k * gts * nbk + ti * nbk + j
                                i = blk_base + rel
                                oh = ohpool.tile([P, P], F16, tag="oh")
                                if mode != "nooh":
                                    nc.any.tensor_scalar(
                                        oh[:],
                                        iota_h[:],
                                        rowm_sb[:, i : i + 1],
                                        valm_sb[:, i : i + 1],
                                        mybir.AluOpType.is_equal,
                                        mybir.AluOpType.mult,
                                    )
                                nc.tensor.matmul(
                                    psa[:],
                                    lhsT=oh[:],
                                    rhs=g[:, rel, :],
                                    start=(mm == 0),
                                    stop=(mm == nb - 1),
                                )
                                mm += 1

                        ah_sb = apool.tile([P, D], F32, tag="ah")
                        nc.any.tensor_copy(ah_sb[:], psa[:, 0:D])
                        a_sb = apool.tile([P, D], F32, tag="a")
                        nc.vector.tensor_tensor(
                            a_sb[:], ah_sb[:], psa[:, D : 2 * D], mybir.AluOpType.add
                        )

                        pst = pst_pool.tile([P, P], F32, tag="pst")
                        nc.tensor.transpose(pst[:], a_sb[:], ident[:])
                        at_sb = atpool.tile([P, P], F32, tag="at")
                        nc.any.tensor_copy(at_sb[:], pst[:])

                        xt = xtpool.tile([P, P], F32, tag="xt")
                        nc.sync.dma_start(xt[:], ftT[:, t * P : (t + 1) * P])

                        pso = pso_pool.tile([P, D], F32, tag="pso")
                        nc.tensor.matmul(
                            pso[:], lhsT=at_sb[:], rhs=wn_sb[:], start=True, stop=False
                        )
                        nc.tensor.matmul(
                            pso[:], lhsT=xt[:], rhs=ws_sb[:], start=False, stop=True
                        )

                        o_sb = opool.tile([P, D], F32, tag="o")
                        nc.any.tensor_copy(o_sb[:], pso[:])
                        rows = P if t < n_tiles - 1 else rows_last
                        nc.sync.dma_start(out[t * P : t * P + rows, :], o_sb[:rows, :])

                    blk_base += g_nblk

            if repeat > 1:
                with tc.For_i(0, repeat, 1):
                    emit_body()
            else:
                emit_body()

    nc.compile()
    return nc


def host_prep(features, edge_row, edge_col, edge_vals, n_cores, nodes_per_core,
              bucket_rows, gt):
    n_nodes = features.shape[0]
    features = np.ascontiguousarray(np.asarray(features, dtype=np.float32))
    edge_row = np.asarray(edge_row).astype(np.int64)
    edge_col = np.asarray(edge_col).astype(np.int64)
    edge_vals = np.asarray(edge_vals, dtype=np.float32)

    hi = features.astype(np.float16)
    lo = (features - hi.astype(np.float32)).astype(np.float16)
    fpair = np.concatenate([hi, lo], axis=1)  # [N, 256] f16

    n_tiles = (nodes_per_core + P - 1) // P
    rows_last = nodes_per_core - (n_tiles - 1) * P
    n_buckets = (n_nodes + bucket_rows - 1) // bucket_rows
    groups = plan_groups(n_tiles, gt)

    core_lo = np.searchsorted(edge_row, np.arange(n_cores) * nodes_per_core, "left")
    core_hi = np.searchsorted(
        edge_row, (np.arange(n_cores) + 1) * nodes_per_core, "left"
    )

    # uniform per-(tile,bucket) block budget across cores
    nbk = 1
    percore = []
    for m in range(n_cores):
        s, e = core_lo[m], core_hi[m]
        rows = (edge_row[s:e] - m * nodes_per_core).astype(np.int64)
        cols = edge_col[s:e]
        tile_of = rows >> 7
        buck_of = cols // bucket_rows
        key = tile_of * n_buckets + buck_of
        cnt = np.bincount(key, minlength=n_tiles * n_buckets)
        if cnt.size:
            nbk = max(nbk, int((cnt.max() + P - 1) // P))
        percore.append((rows, cols, edge_vals[s:e], key))

    nb = n_buckets * nbk
    nblk_total = n_tiles * nb
    slots_total = nblk_total * P

    # static slot base of each (tile, bucket) section, following the group
    # ordering: [group][bucket][tile-in-group][block j][partition]
    sect_base = np.zeros((n_tiles, n_buckets), np.int64)
    blk_base = 0
    for t0, gts in groups:
        for k in range(n_buckets):
            for ti in range(gts):
                sect_base[t0 + ti, k] = (blk_base + k * gts * nbk + ti * nbk) * P
        blk_base += gts * nb

    core_maps = []
    for m in range(n_cores):
        rows, cols, vals, key = percore[m]
        order = np.argsort(key, kind="stable")
        rows_s, cols_s, vals_s, key_s = (
            rows[order], cols[order], vals[order], key[order],
        )
        starts = np.searchsorted(key_s, np.arange(n_tiles * n_buckets))
        pos = np.arange(rows_s.size, dtype=np.int64) - starts[key_s]
        slot = sect_base[key_s // n_buckets, key_s % n_buckets] + pos

        slotvals = np.zeros(slots_total, np.int16)
        slotvals[slot] = (cols_s % bucket_rows).astype(np.int16)
        mask = np.zeros(slots_total, bool)
        mask[slot] = True
        lastreal = np.maximum.accumulate(np.where(mask, np.arange(slots_total), 0))
        slotvals = slotvals[lastreal]
        idx16 = slotvals.reshape(slots_total // 16, 16).T.copy()
        idxm = np.tile(idx16, (8, 1))

        rowm = np.zeros((P, nblk_total), np.float32)
        valm = np.zeros((P, nblk_total), np.float32)
        rowm[slot % P, slot // P] = (rows_s & 127).astype(np.float32)
        valm[slot % P, slot // P] = vals_s

        ftT = np.zeros((D, n_tiles * P), np.float32)
        lo_n = m * nodes_per_core
        hi_n = min(lo_n + n_tiles * P, n_nodes)
        ftT[:, : hi_n - lo_n] = features[lo_n:hi_n].T

        core_maps.append({"idxm": idxm, "rowm": rowm, "valm": valm, "ftT": ftT})

    return fpair, core_maps, nbk, n_tiles, rows_last


_PROGRAM_CACHE = {}


def _get_program(key_args):
    if key_args not in _PROGRAM_CACHE:
        _PROGRAM_CACHE[key_args] = build_program(*key_args)
    return _PROGRAM_CACHE[key_args]


def prepare(features, edge_row, edge_col, edge_vals, weight_neigh, weight_self,
            n_cores=8, bucket_rows=25000, gt=6):
    """Host prep + program build; returns (nc, in_maps, n_nodes)."""
    n_nodes = int(np.asarray(features).shape[0])
    nodes_per_core = (n_nodes + n_cores - 1) // n_cores
    fpair, core_maps, nbk, n_tiles, rows_last = host_prep(
        features, edge_row, edge_col, edge_vals, n_cores, nodes_per_core,
        bucket_rows, gt,
    )
    nc = _get_program(
        (n_cores, n_tiles, nbk, rows_last, fpair.shape[0], bucket_rows, gt)
    )
    wnp = np.ascontiguousarray(np.asarray(weight_neigh, dtype=np.float32))
    wsp = np.ascontiguousarray(np.asarray(weight_self, dtype=np.float32))
    in_maps = []
    for m in range(n_cores):
        im = {"fpair": fpair, "wn": wnp, "ws": wsp}
        im.update(core_maps[m])
        in_maps.append(im)
    return nc, in_maps, n_nodes


def run(features, edge_row, edge_col, edge_vals, weight_neigh, weight_self,
        n_cores=8, bucket_rows=25000, gt=6):
    nc, in_maps, n_nodes = prepare(
        features, edge_row, edge_col, edge_vals, weight_neigh, weight_self,
        n_cores, bucket_rows, gt,
    )
    res = run_bass_kernel_spmd(nc, in_maps, core_ids=list(range(n_cores)))
    out = np.concatenate([res.results[m]["out"] for m in range(n_cores)], axis=0)
    return out[:n_nodes]


def kernel(**inputs):
    return run(
        inputs["features"],
        inputs["edge_row"],
        inputs["edge_col"],
        inputs["edge_vals"],
        inputs["weight_neigh"],
        inputs["weight_self"],
    )

